# revision 1
# baseline (speedup 1.0000x reference)
"""Trainium2 Bass kernel V4 for nn_CNN_3496103379215.

V3 + latency trims:
- no halo memsets: image-edge tiles narrow the matmul contraction window
  instead (tile 0 contracts partitions [2:128], last tile [0:34]) — exact,
  and the input DMAs start with no DVE dependency.
- first tile's conf/data loads split into x-halves so the first den matmuls
  start after ~1/4 of the load; 2 warm-up matmuls on the T tile ramp the PE
  p-state while loads finish.
- bias add on DVE (tensor_scalar_add): the Act engine only ever runs Copy,
  so no activation-table reloads.
- final tile runs y-stage/epilogue per x-half so the post-PE tail is short.
"""

import os
import numpy as np
from contextlib import ExitStack

EPS = 1e-20
H = W = 1024
HP = WP = 512
PER_CORE = 2
N_CORES = 8
TILE_ROWS = 124
N_TILES = 9

_CACHE = {}


def _host_tensors(weight, bias):
    weight = np.asarray(weight, np.float32)
    bias = np.asarray(bias, np.float32)
    wsum = weight.sum(axis=(1, 2, 3))
    s = (0.25 / (wsum + EPS)).astype(np.float32)
    # blocks 0..9: standard band (partition p = input row r0+p, r0 = ys-2);
    # blocks 10..19: tile-0 variant, band shifted up 2 with top taps clipped
    # (zero padding), so the contraction stays at base partition 0.
    tm = np.zeros((128, 20 * 128), np.float32)
    js = np.arange(62)
    for shift in range(2):
        for dx in range(5):
            for par in range(2):
                T = np.zeros((128, 128), np.float32)
                for dy in range(5):
                    r = 2 * js + par + dy - 2 * shift
                    ok = r >= 0
                    T[r[ok], js[ok]] = weight[0, 0, dy, dx] * s[0]
                    T[r[ok], 64 + js[ok]] = weight[1, 0, dy, dx] * s[1]
                i = shift * 10 + par * 5 + dx
                tm[:, i * 128:(i + 1) * 128] = T
    bcol = np.zeros((128, 1), np.float32)
    bcol[0:62, 0] = bias[0]
    bcol[64:126, 0] = bias[1]
    return tm, bcol


def _host_tail(weight, bias):
    """Merged final tile: img0 rows 990..1023 at partitions 0..33, img1 at
    34..67 (gapless, so one base-0 contraction covers both).  Stationary [128,64]: cols 0..15 img0-ch0, 16..31 img0-ch1,
    32..47 img1-ch0, 48..63 img1-ch1 (16 pooled... 16 output-row pairs each).
    Returns tmt [128, 10*64] and the matching bias column."""
    weight = np.asarray(weight, np.float32)
    bias = np.asarray(bias, np.float32)
    wsum = weight.sum(axis=(1, 2, 3))
    s = (0.25 / (wsum + EPS)).astype(np.float32)
    tmt = np.zeros((128, 10 * 64), np.float32)
    js = np.arange(16)
    for dx in range(5):
        for par in range(2):
            T = np.zeros((128, 64), np.float32)
            for dy in range(5):
                r = 2 * js + par + dy
                ok = r <= 33            # rows >= 34 are below the image: zero pad
                T[r[ok], js[ok]] = weight[0, 0, dy, dx] * s[0]
                T[r[ok], 16 + js[ok]] = weight[1, 0, dy, dx] * s[1]
                T[34 + r[ok], 32 + js[ok]] = weight[0, 0, dy, dx] * s[0]
                T[34 + r[ok], 48 + js[ok]] = weight[1, 0, dy, dx] * s[1]
            i = dx * 2 + par
            tmt[:, i * 64:(i + 1) * 64] = T
    bt = np.zeros((128, 1), np.float32)
    for k, ch in enumerate((0, 1, 0, 1)):
        bt[16 * k:16 * (k + 1), 0] = bias[ch]
    return tmt, bt


def _build_program(repeat=1):
    import concourse.bass as bass
    import concourse.tile as tile
    from concourse import bacc, mybir

    f32 = mybir.dt.float32
    f32r = mybir.dt.float32r
    u8 = mybir.dt.uint8
    nc = bacc.Bacc("TRN2", target_bir_lowering=False)

    TMW = 20 * 128 + 10 * 64
    data_ext = nc.declare_dram_parameter("data", [PER_CORE, 1, H, W], f32r, isOutput=False)
    conf_ext = nc.declare_dram_parameter("conf", [PER_CORE, 1, H, W], f32r, isOutput=False)
    tm_ext = nc.declare_dram_parameter("tm", [128, TMW], f32r, isOutput=False)
    bcol_ext = nc.declare_dram_parameter("bcol", [128, 2], f32, isOutput=False)
    x1_ext = nc.declare_dram_parameter("x1", [PER_CORE, 2, HP, WP], f32, isOutput=True)
    c1_ext = nc.declare_dram_parameter("c1", [PER_CORE, 2, HP, WP], f32, isOutput=True)

    gt = mybir.AluOpType.is_gt
    mx = mybir.AluOpType.max
    dv = mybir.AluOpType.divide

    with tile.TileContext(nc) as tc, ExitStack() as ctx:
        consts = ctx.enter_context(tc.tile_pool(name="consts", bufs=1))
        inp = ctx.enter_context(tc.tile_pool(name="inp", bufs=3))
        psum = ctx.enter_context(tc.tile_pool(name="psum", bufs=1, space="PSUM"))
        sx = ctx.enter_context(tc.tile_pool(name="sx", bufs=3))

        # shifted blocks (tile 0's stationary) load first so the first
        # matmuls and the PE warm-up only wait ~1.8us; standard and tail
        # blocks follow after the first x-half input loads.
        tm_t = consts.tile([128, TMW], f32r)
        nc.sync.dma_start(out=tm_t[:, 10 * 128:15 * 128], in_=tm_ext[:, 10 * 128:15 * 128])
        bcol_t = consts.tile([128, 2], f32)
        nc.sync.dma_start(out=bcol_t[:, :], in_=bcol_ext[:, :])

        def tsl(dx, par, shift=0):
            i = shift * 10 + par * 5 + dx
            return tm_t[:, i * 128:i * 128 + 128]

        def tsl_tail(dx, par):
            i = dx * 2 + par
            return tm_t[:, 20 * 128 + i * 64:20 * 128 + i * 64 + 64]

        # tiles 0..7 per image cover out rows 0..991; the merged tail tile
        # covers rows 992..1023 of BOTH images (img0 at partitions 0..33,
        # img1 at 64..97; output groups of 16 partition rows per (img, ch)).
        TAIL = (-1, -1)

        WP4 = W + 4   # 2 zero halo columns each side; image cols at [2, 1026)

        def issue_loads(ent, split=False):
            img, t = ent
            conf_t = inp.tile([128, WP4], f32r, tag="conf")
            data_t = inp.tile([128, WP4], f32r, tag="data")
            for tt in (conf_t, data_t):
                # zero the 2+2 halo columns (f32 view: no f32r memset ISA op)
                nc.gpsimd.memset(tt[:, 0:2].bitcast(f32), 0.0)
                nc.gpsimd.memset(tt[:, 1026:1028].bitcast(f32), 0.0)
            if ent == TAIL:
                for p0, im in ((0, 0), (34, 1)):
                    nc.sync.dma_start(out=conf_t[p0:p0 + 34, 2:1026],
                                      in_=conf_ext[im, 0, 990:1024, :])
                    nc.sync.dma_start(out=data_t[p0:p0 + 34, 2:1026],
                                      in_=data_ext[im, 0, 990:1024, :])
                return conf_t, data_t
            ys = TILE_ROWS * t
            cr0 = 0 if t == 0 else ys - 2
            cr1 = cr0 + (126 if t == 0 else 128)
            if split:
                for xs, xe in ((0, 516), (516, W)):
                    nc.sync.dma_start(out=conf_t[0:cr1 - cr0, 2 + xs:2 + xe],
                                      in_=conf_ext[img, 0, cr0:cr1, xs:xe])
                    nc.sync.dma_start(out=data_t[0:cr1 - cr0, 2 + xs:2 + xe],
                                      in_=data_ext[img, 0, cr0:cr1, xs:xe])
            else:
                nc.sync.dma_start(out=conf_t[0:cr1 - cr0, 2:1026],
                                  in_=conf_ext[img, 0, cr0:cr1, :])
                nc.sync.dma_start(out=data_t[0:cr1 - cr0, 2:1026],
                                  in_=data_ext[img, 0, cr0:cr1, :])
            return conf_t, data_t

        def issue_dc(tiles, ent, split=False):
            # halo cols of conf/data are zero, so dc halos come out zero too
            conf_t, data_t = tiles
            dc_t = inp.tile([128, WP4], f32r, tag="dc")
            if ent == TAIL:
                nc.gpsimd.tensor_mul(dc_t[0:68, :],
                                     data_t[0:68, :], conf_t[0:68, :])
                return dc_t
            khi = 126 if ent[1] == 0 else 128
            if split:
                # first tile: DVE is idle at startup and ~2x faster per element
                nc.vector.tensor_mul(dc_t[0:khi, 0:518],
                                     data_t[0:khi, 0:518], conf_t[0:khi, 0:518])
                nc.vector.tensor_mul(dc_t[0:khi, 518:WP4],
                                     data_t[0:khi, 518:WP4], conf_t[0:khi, 518:WP4])
            else:
                nc.gpsimd.tensor_mul(dc_t[0:khi, :],
                                     data_t[0:khi, :], conf_t[0:khi, :])
            return dc_t

        for _rep in range(repeat):
          # tail tile mid-sequence: its 16-store burst overlaps img1's PE
          # work, and the final tile is a regular one with a 2-store trail
          seq = [(0, t) for t in range(N_TILES - 1)] + [TAIL] + \
                [(1, t) for t in range(N_TILES - 1)]
          cur = issue_loads(seq[0], split=True)
          nc.sync.dma_start(out=tm_t[:, 15 * 128:20 * 128], in_=tm_ext[:, 15 * 128:20 * 128])
          nc.sync.dma_start(out=tm_t[:, 0:10 * 128], in_=tm_ext[:, 0:10 * 128])
          nc.sync.dma_start(out=tm_t[:, 20 * 128:], in_=tm_ext[:, 20 * 128:])
          cur_dc = issue_dc(cur, seq[0], split=True)
          nxt = issue_loads(seq[1])
          # warm up the PE p-state while the first loads land (shifted T
          # region is the first DMA to finish)
          wrm = psum.tile([128, 512], f32, tag="denE0")
          for w in range(2):
              nc.tensor.matmul(wrm[0:128, 0:512], tsl(0, 0, shift=1),
                               tm_t[:, 10 * 128:10 * 128 + 512],
                               start=True, stop=True)
          for i, ent in enumerate(seq):
            conf_t, data_t = cur
            dc_t = cur_dc
            tail = ent == TAIL
            img, t = ent
            last = i + 1 == len(seq)
            if tail:
                np_, pr0, khi, prng = 64, 496, 68, 16
                groups = ((0, 0, 0), (0, 1, 16), (1, 0, 32), (1, 1, 48))
            else:
                np_, pr0, khi, prng = 128, TILE_ROWS * t // 2, (126 if t == 0 else 128), 62
                groups = ((img, 0, 0), (img, 1, 64))

            ceE = sx.tile([128, 512], f32, tag="ceE")
            ceO = sx.tile([128, 512], f32, tag="ceO")
            nxE = sx.tile([128, 512], f32, tag="nxE")
            nxO = sx.tile([128, 512], f32, tag="nxO")
            cxE = sx.tile([128, 512], f32, tag="cxE")
            cxO = sx.tile([128, 512], f32, tag="cxO")
            mE = sx.tile([128, 512], u8, tag="mE")
            mO = sx.tile([128, 512], u8, tag="mO")
            my = sx.tile([128, 512], u8, tag="my")
            x1s = sx.tile([128, 512], f32, tag="x1s")

            def store(dst_ext, strip, h):
                for im, ch, p0 in groups:
                    nc.sync.dma_start(out=dst_ext[im, ch, pr0:pr0 + prng, h],
                                      in_=strip[p0:p0 + prng, h])

            def ystage_early(h):
                # depends only on den results: y-compare, c1 stores, 1/c1
                nc.vector.tensor_tensor(my[0:np_, h], cxO[0:np_, h], cxE[0:np_, h], op=gt)
                nc.vector.tensor_tensor(cxE[0:np_, h], cxE[0:np_, h], cxO[0:np_, h], op=mx)
                nc.vector.reciprocal(x1s[0:np_, h], cxE[0:np_, h])
                store(c1_ext, cxE, h)

            def ystage_late(h):
                bc = bcol_t[0:np_, 1:2] if tail else bcol_t[0:np_, 0:1]
                nc.vector.copy_predicated(nxE[0:np_, h], my[0:np_, h], nxO[0:np_, h])
                if last:
                    nc.vector.tensor_mul(x1s[0:np_, h], x1s[0:np_, h], nxE[0:np_, h])
                    nc.vector.tensor_scalar_add(x1s[0:np_, h], x1s[0:np_, h], bc)
                else:
                    nc.gpsimd.tensor_mul(x1s[0:np_, h], x1s[0:np_, h], nxE[0:np_, h])
                    nc.scalar.activation(x1s[0:np_, h], x1s[0:np_, h],
                                         mybir.ActivationFunctionType.Identity,
                                         bias=bc, scale=1.0)
                    # x1 stores issue from the Act queue right after the bias
                    # lands there, so their sem-wait never blocks SP's loads
                    for im, ch, p0 in groups:
                        nc.scalar.dma_start(out=x1_ext[im, ch, pr0:pr0 + prng, h],
                                            in_=x1s[p0:p0 + prng, h])
                    return
                if last:
                    # parallel-issue the two final stores from SP and Act
                    im, ch, p0 = groups[0]
                    nc.sync.dma_start(out=x1_ext[im, ch, pr0:pr0 + prng, h],
                                      in_=x1s[p0:p0 + prng, h])
                    im, ch, p0 = groups[1]
                    nc.scalar.dma_start(out=x1_ext[im, ch, pr0:pr0 + prng, h],
                                        in_=x1s[p0:p0 + prng, h])
                else:
                    store(x1_ext, x1s, h)

            for c in (0, 1):
                denE = psum.tile([128, 512], f32, tag=f"denE{c}")
                denO = psum.tile([128, 512], f32, tag=f"denO{c}")
                nomE = psum.tile([128, 512], f32, tag=f"nomE{c}")
                nomO = psum.tile([128, 512], f32, tag=f"nomO{c}")
                for bank, rhs, par in ((denE, conf_t, 0), (denO, conf_t, 1),
                                       (nomE, dc_t, 0), (nomO, dc_t, 1)):
                    for k, dx in enumerate(range(5)):
                        src0 = c * 512 + dx
                        if tail:
                            T = tsl_tail(dx, par)
                        else:
                            T = tsl(dx, par, shift=1 if t == 0 else 0)
                        nc.tensor.matmul(bank[0:np_, 0:512],
                                         T[0:khi, :],
                                         rhs[0:khi, src0:src0 + 512],
                                         start=(k == 0), stop=(k == 4))

                if c == 0 and i + 2 < len(seq):
                    nxt2 = issue_loads(seq[i + 2])

                dEv = denE.rearrange("p (x two) -> p x two", two=2)
                dOv = denO.rearrange("p (x two) -> p x two", two=2)
                nEv = nomE.rearrange("p (x two) -> p x two", two=2)
                nOv = nomO.rearrange("p (x two) -> p x two", two=2)
                h = slice(c * 256, c * 256 + 256)

                nc.scalar.copy(ceE[0:np_, h], dEv[0:np_, :, 0])
                nc.scalar.copy(ceO[0:np_, h], dOv[0:np_, :, 0])
                nc.vector.tensor_tensor(mE[0:np_, h], dEv[0:np_, :, 1], ceE[0:np_, h], op=gt)
                nc.vector.tensor_tensor(mO[0:np_, h], dOv[0:np_, :, 1], ceO[0:np_, h], op=gt)
                nc.vector.tensor_tensor(cxE[0:np_, h], ceE[0:np_, h], dEv[0:np_, :, 1], op=mx)
                nc.vector.tensor_tensor(cxO[0:np_, h], ceO[0:np_, h], dOv[0:np_, :, 1], op=mx)
                if last:
                    ystage_early(h)
                nc.scalar.copy(nxE[0:np_, h], nEv[0:np_, :, 0])
                nc.scalar.copy(nxO[0:np_, h], nOv[0:np_, :, 0])
                nc.vector.copy_predicated(nxE[0:np_, h], mE[0:np_, h], nEv[0:np_, :, 1])
                nc.vector.copy_predicated(nxO[0:np_, h], mO[0:np_, h], nOv[0:np_, :, 1])

                if last:
                    ystage_late(h)

            if not last:
                ystage_early(slice(0, 512))
                ystage_late(slice(0, 512))
                nxt_dc = issue_dc(nxt, seq[i + 1])
                cur, cur_dc = nxt, nxt_dc
                if i + 2 < len(seq):
                    nxt = nxt2
    nc.compile()
    return nc


def modeled_time_ns():
    """TimelineSim-modeled per-core duration of the compiled program (the
    local stand-in for the profiled HW exec time when no NTFF hook exists)."""
    try:
        nc = _CACHE.get(("nc", 1)) or _build_program(1)
        _CACHE[("nc", 1)] = nc
        from concourse.timeline_sim import TimelineSim
        return int(TimelineSim(nc, no_exec=True).simulate())
    except Exception:
        return None


def kernel(data, conf, weight, bias):
    from concourse.bass_utils import run_bass_kernel_spmd

    data = np.ascontiguousarray(np.asarray(data, np.float32))
    conf = np.ascontiguousarray(np.asarray(conf, np.float32))
    repeat = int(os.environ.get("BASS_KERNEL_REPEAT", "1"))
    key = ("nc", repeat)
    if key not in _CACHE:
        _CACHE[key] = _build_program(repeat)
    nc = _CACHE[key]

    tm, bcol = _host_tensors(weight, bias)
    tmt, btail = _host_tail(weight, bias)
    tm_full = np.concatenate([tm, tmt], axis=1)
    bcol_full = np.concatenate([bcol, btail], axis=1)
    in_maps = []
    for c in range(N_CORES):
        sl = slice(c * PER_CORE, (c + 1) * PER_CORE)
        in_maps.append({"data": data[sl], "conf": conf[sl],
                        "tm": tm_full, "bcol": bcol_full})

    trace = bool(int(os.environ.get("BASS_KERNEL_TRACE", "0")))
    try:
        res = run_bass_kernel_spmd(nc, in_maps, list(range(N_CORES)), trace=trace)
    except ModuleNotFoundError:
        # BASS_TRACE set in an env without the axon NTFF hook module:
        # retry untraced rather than failing the whole run
        prev = os.environ.get("BASS_NEVER_TRACE")
        os.environ["BASS_NEVER_TRACE"] = "1"
        try:
            res = run_bass_kernel_spmd(nc, in_maps, list(range(N_CORES)), trace=False)
        finally:
            if prev is None:
                os.environ.pop("BASS_NEVER_TRACE", None)
            else:
                os.environ["BASS_NEVER_TRACE"] = prev
    kernel.last_exec_time_ns = res.exec_time_ns

    x1 = np.concatenate([r["x1"] for r in res.results], axis=0)
    c1 = np.concatenate([r["c1"] for r in res.results], axis=0)
    return x1, c1


kernel.last_exec_time_ns = None

